# revision 26
# baseline (speedup 1.0000x reference)
"""Trainium2 Bass kernel for PixelSNAIL-style strict-causal attention.

Problem: query/key/value [B=4, H=64, W=64, C=256] fp32.
  S = 4096 tokens per batch; scores = (Q K^T)/16 with strict causal mask
  (position i attends to j < i); out = softmax(scores) @ V (row 0 -> 0).

Strategy (8 NeuronCores):
  - 2 cores per batch: context-parallel split of the key/value blocks by
    parity (core h owns k-blocks h, h+2, ..., h+30). Every core runs the
    IDENTICAL program (SPMD) over all 32 query blocks of its batch.
  - No max-subtraction in softmax (scores ~ N(0,1), exp is safe in fp32),
    so per-core partial numerators/denominators combine exactly on host.
  - Host pre-transposes Q and K (c-major) and converts Q/K/V to bf16:
    halves HBM traffic (input stream 23.6us -> 12.9us), bf16 weights get
    fast-weight-load so LDWEIGHTS hides under shorter streams, and
    measured per-matmul issue at equal chip clock is slightly better than
    fp32r. Matmuls accumulate fp32 in PSUM; quantization error ~2e-3 rel,
    far under the 2e-2 gate. V gets a ones-column so the softmax
    denominator accumulates in PSUM alongside the numerator.
  - Strict-causal/diagonal masking is data-driven: an fp32 additive mask
    applied only on each q-slot's diagonal position-pair. The mask rides
    the scalar HWDGE queue FIRST so it lands ~8us in (a late mask stalls
    slot 7's diagonal pair and re-throttles the PE clock gate).
  - Output is bf16 (halves the output drain): per slot the four PSUM O
    tiles drain via VectorE (qs 0,1) + ScalarE (qs 2,3) into one bf16
    SBUF tile shipped by a single 3D-AP DMA on sync.

Layout per core (b = core//2, h = core%2):
  qt_in [256, 4096] bf16 = Q[b]^T
  kt_in [256, 2048] bf16 = K[b][blocks h::2]^T
  v_in  [2048, 258] bf16 = V[b][blocks h::2] ++ ones column
  m_in  [128, 768] fp32  = additive mask for the diagonal position-pair
  o_out [4096, 258] bf16 = partial (numerator ++ denominator) for this core

Program: slot order 7,6,5,4,3,2,0,1 -- descending so the compute-heavy
  slot overlaps the input streaming, with the two tiny slots (0: 1 pair,
  1: 2 pairs) sandwiched after slot 2: their serial St->mask->exp latency
  is absorbed by the neighbouring slots' PE work instead of stalling a
  bare pipeline tail. Big slots (p>=5) iterate pairs in kt-arrival order
  (diagonal last); small slots run the diagonal pair first (its longer
  chain hides the previous slot's O-bank drain). A bf16 warmup burst plus
  filler matmuls in slot 7 keep the PE clock gate open through the
  DMA-paced opening.
"""

import numpy as np

B = 4
S = 4096          # 64*64 tokens per batch
C = 256
NBLK = 32         # 128-row k blocks per batch
NPOS = 16         # k blocks per core (parity split)
NSLOT = 8         # q slots of 512 rows
SCALE = 1.0 / 16.0
NEG = -1.0e30

_CACHE = {}


def _build_nc():
    import concourse.bacc as bacc
    import concourse.mybir as mybir
    import concourse.tile as tile

    F32 = mybir.dt.float32
    BF16 = mybir.dt.bfloat16

    nc = bacc.Bacc("TRN2", target_bir_lowering=False, debug=False, num_devices=8)
    qt_in = nc.dram_tensor("qt_in", [C, S], BF16, kind="ExternalInput").ap()
    kt_in = nc.dram_tensor("kt_in", [C, NPOS * 128], BF16, kind="ExternalInput").ap()
    v_in = nc.dram_tensor("v_in", [NPOS * 128, 258], BF16, kind="ExternalInput").ap()
    m_in = nc.dram_tensor("m_in", [128, 768], F32, kind="ExternalInput").ap()
    o_out = nc.dram_tensor("o_out", [S, 258], BF16, kind="ExternalOutput").ap()

    with tile.TileContext(nc) as tc:
        with (
            tc.tile_pool(name="const", bufs=1) as const,
            tc.tile_pool(name="pt", bufs=3) as ptp,
            tc.tile_pool(name="ms", bufs=2) as msp,
            tc.tile_pool(name="osb", bufs=2) as osbp,
            tc.tile_pool(name="st", bufs=2, space="PSUM") as stp,
            tc.tile_pool(name="op", bufs=4, space="PSUM") as opp,
        ):
            # PE warmup: ~8us of tiny matmuls on memset data, issued during
            # the DMA preamble so the HAM clock gate opens before real work.
            wu = const.tile([128, 64], BF16, tag="wu")
            nc.gpsimd.memset(wu[:], 0.0)
            wu_ps = stp.tile([128, 512], F32, tag="st", name="wu_ps")
            for _ in range(75):
                nc.tensor.matmul(wu_ps[0:64, 0:64], lhsT=wu[:], rhs=wu[:],
                                 start=True, stop=True)

            qt = [const.tile([128, S], BF16, tag=f"qt{c}", name=f"qt{c}") for c in range(2)]
            kt = [
                const.tile([128, NPOS * 128], BF16, tag=f"kt{c}", name=f"kt{c}")
                for c in range(2)
            ]
            vsb = const.tile([128, NPOS * 258], BF16, tag="v")
            mask = const.tile([128, 768], F32, tag="m")

            # DMA placement: qt/kt ride the sync HWDGE ring in consumption
            # order (first-pair chunks first); mask + v pair 0 ride the
            # scalar HWDGE ring (fast, and ACT's first exp isn't until
            # ~12us); the remaining v positions ride gpsimd SWDGE.
            # Per-slot outputs ride sync, queued behind the inputs.
            def qt_dma(c, c0, c1):
                nc.sync.dma_start(qt[c][:, c0:c1], qt_in[c * 128:(c + 1) * 128, c0:c1])

            def kt_dma(c, c0, c1):
                nc.sync.dma_start(kt[c][:, c0:c1], kt_in[c * 128:(c + 1) * 128, c0:c1])

            def v_dma(eng, pos, npos):
                # npos positions in one trigger via a 3D AP
                eng.dma_start(
                    vsb[:, pos * 258:(pos + npos) * 258].rearrange(
                        "p (t v) -> p t v", t=npos
                    ),
                    v_in[pos * 128:(pos + npos) * 128, :].rearrange(
                        "(t p) v -> p t v", p=128
                    ),
                )

            # First-pair loads split across BOTH HWDGE rings so their
            # trigger+receipt latencies run in parallel: kt pair 0 + v pair
            # 0 on scalar, slot-7 queries first on sync. Mask rides gpsimd
            # (first trigger there; not needed until ~25us).
            for c in range(2):
                nc.scalar.dma_start(
                    kt[c][:, 0:256], kt_in[c * 128:(c + 1) * 128, 0:256]
                )                                     # k pair 0
            # Slot-7 queries split across rings: c0 on sync, c1 on scalar --
            # both land ~11.3us instead of c1 waiting behind c0's receipt
            # on a single ring. kt pairs 1-2 move up to sync slots #2-3.
            qt_dma(0, 7 * 512, 8 * 512)
            nc.scalar.dma_start(
                qt[1][:, 7 * 512:8 * 512],
                qt_in[128:256, 7 * 512:8 * 512],
            )
            v_dma(nc.scalar, 0, 2)
            nc.gpsimd.dma_start(mask[:], m_in[:])
            for c in range(2):
                kt_dma(c, 256, 768)                   # k pairs 1-2
            for c in range(2):
                kt_dma(c, 768, 2048)                  # k pairs 3-7
            v_dma(nc.scalar, 2, 2)
            v_dma(nc.scalar, 4, 4)
            for c in range(2):
                qt_dma(c, 6 * 512, 7 * 512)           # slot 6 queries
            v_dma(nc.gpsimd, 8, 8)
            for c in range(2):
                qt_dma(c, 4 * 512, 6 * 512)           # slots 5-4
            for c in range(2):
                qt_dma(c, 0, 4 * 512)                 # slots 3-0

            def emit_drain(o_ps, p):
                # O drain: split across VectorE and ScalarE into one bf16
                # SBUF tile, shipped by 3D-AP DMA(s) on sync. Emission is
                # DEFERRED into the next slot (after its diagonal pair's
                # mask-add + exp) so the mask TENSOR_TENSOR sits at the head
                # of the strict-FIFO DVE queue instead of behind these casts.
                ob = osbp.tile([128, 4 * 258], BF16, tag="ob", name=f"ob{p}")
                for qs in range(4):
                    eng = nc.vector.tensor_copy if qs < 2 else nc.scalar.copy
                    eng(ob[:, qs * 258:(qs + 1) * 258], o_ps[qs][:])
                if p == 1:
                    # Final slot processed: ship each half as soon as its
                    # copy engine finishes, so the last (small) DMA's HBM
                    # write receipt starts earlier.
                    for half in range(2):
                        nc.sync.dma_start(
                            o_out[p * 512 + half * 256:p * 512 + (half + 1) * 256, :]
                            .rearrange("(t p) v -> p t v", p=128),
                            ob[:, half * 516:(half + 1) * 516].rearrange(
                                "p (t v) -> p t v", t=2
                            ),
                        )
                else:
                    nc.sync.dma_start(
                        o_out[p * 512:(p + 1) * 512, :].rearrange(
                            "(t p) v -> p t v", p=128
                        ),
                        ob[:].rearrange("p (t v) -> p t v", t=4),
                    )

            pending_drain = None
            for p in (7, 6, 5, 4, 3, 2, 0, 1):
                o_ps = None
                # Slot 7 (DMA-paced) iterates pairs in kt-arrival order with
                # the masked pair last; all later slots run the masked pair
                # first so its longer St->mask->exp chain overlaps the
                # previous slot's O-bank drain.
                if p == NSLOT - 1:
                    t_order = list(range(p + 1))
                else:
                    t_order = [p] + list(range(p))
                for ti, t in enumerate(t_order):
                    last = t == p
                    first_it = ti == 0
                    last_it = ti == len(t_order) - 1
                    # On the masked pair, position jp=1 is fully blocked for
                    # q-sub-blocks 0,1 on both cores: compute only the live
                    # 256-column half.
                    width = 768 if last else 1024
                    st = stp.tile([128, 1024], F32, tag="st", name=f"st{p}_{t}")
                    if p == NSLOT - 1 and t > 0:
                        # Keep the PE clock gate open during the DMA-paced
                        # streaming phase: tiny filler matmuls into this pair's
                        # St region, overwritten by the real start=True matmul.
                        # Pair 0 gets NONE: its data lands right as the warmup
                        # burst ends, and fillers ahead of it in the in-order
                        # tensor queue only delay the first real matmul.
                        for _ in range(24 if t < 3 else 8):
                            nc.tensor.matmul(
                                st[0:64, 0:64], lhsT=wu[:], rhs=wu[:],
                                start=True, stop=True,
                            )
                    for jp in range(2):
                        pos = 2 * t + jp
                        qoff = p * 512 + (256 if (last and jp == 1) else 0)
                        n = 256 if (last and jp == 1) else 512
                        for c in range(2):
                            nc.tensor.matmul(
                                st[:, jp * 512:jp * 512 + n],
                                lhsT=kt[c][:, pos * 128:(pos + 1) * 128],
                                rhs=qt[c][:, qoff:qoff + n],
                                start=(c == 0),
                                stop=(c == 1),
                            )
                    pt = ptp.tile([128, 1024], BF16, tag="pt", name=f"pt{p}_{t}")
                    if last:
                        # Masked-score sum goes to SBUF scratch, not back
                        # into the St PSUM bank: a same-bank read+write on
                        # DVE runs at half rate (~966ns for 768 cols vs
                        # ~490ns to SBUF), and this add sits on the slot
                        # boundary critical path.
                        ms = msp.tile([128, 768], F32, tag="ms", name=f"ms{p}")
                        nc.vector.tensor_tensor(
                            ms[:, :width], st[:, :width], mask[:, :width],
                            mybir.AluOpType.add,
                        )
                        nc.scalar.activation(
                            pt[:, :width], ms[:, :width],
                            mybir.ActivationFunctionType.Exp, scale=SCALE,
                        )
                    else:
                        nc.scalar.activation(
                            pt[:, :width], st[:, :width],
                            mybir.ActivationFunctionType.Exp, scale=SCALE,
                        )
                    if first_it:
                        # Emit the previous slot's deferred drain now (its
                        # copies queue AFTER this pair's mask-add and exp on
                        # the DVE/ACT rings), then allocate this slot's O
                        # tiles so the pool's write-after-read ordering sees
                        # those copies first.
                        if pending_drain is not None:
                            emit_drain(*pending_drain)
                            pending_drain = None
                        o_ps = [
                            opp.tile([128, 258], F32, tag="o", name=f"o_ps{p}_{qs}")
                            for qs in range(4)
                        ]
                    # qs-outer so each o_ps[qs] takes its final write as
                    # early as possible -- its drain copy can then start
                    # before the pair's remaining PV matmuls finish.
                    for qs in range(4):
                        for jp in range(2):
                            if last and jp == 1 and qs < 2:
                                continue
                            loff = jp * 512 + qs * 128
                            if last and jp == 1:
                                loff = 512 + (qs - 2) * 128
                            nc.tensor.matmul(
                                o_ps[qs][:],
                                lhsT=pt[:, loff:loff + 128],
                                rhs=vsb[:, (2 * t + jp) * 258:(2 * t + jp + 1) * 258],
                                start=(first_it and jp == 0),
                                stop=(last_it and jp == (0 if (last and qs < 2) else 1)),
                            )
                pending_drain = (o_ps, p)
            emit_drain(*pending_drain)
    nc.compile()
    return nc


def _get_nc():
    if "nc" not in _CACHE:
        _CACHE["nc"] = _build_nc()
    return _CACHE["nc"]


def _make_masks():
    """Additive masks [128, 768] for the diagonal position-pair of each slot.

    Free-dim layout: (jp in {0,1}) x (qs in {0..3}) x 128. On the diag pair
    t=p, position jp holds k-block 4p + 2*jp + h vs q-sub-block 4p + qs:
      block <  qblock -> fully allowed (0)
      block == qblock -> strict lower-triangular (allowed iff q_local > k_local)
      block >  qblock -> fully blocked (NEG)
    """
    k_loc = np.arange(128)[:, None]
    q_loc = np.arange(128)[None, :]
    strict = np.where(q_loc > k_loc, 0.0, NEG).astype(np.float32)
    zeros = np.zeros((128, 128), np.float32)
    blocked = np.full((128, 128), NEG, np.float32)
    masks = []
    for h in range(2):
        chunks = []
        for jp, qs_list in ((0, (0, 1, 2, 3)), (1, (2, 3))):
            rel = 2 * jp + h  # k-block offset relative to 4p
            for qs in qs_list:
                if rel < qs:
                    chunks.append(zeros)
                elif rel == qs:
                    chunks.append(strict)
                else:
                    chunks.append(blocked)
        masks.append(np.concatenate(chunks, axis=1))
    return masks


def _run(query, key, value, trace=False, trace_cores=None):
    import ml_dtypes
    from concourse.bass_utils import run_bass_kernel_spmd

    BF = ml_dtypes.bfloat16
    query = np.ascontiguousarray(np.asarray(query, dtype=np.float32)).reshape(B, S, C)
    key = np.ascontiguousarray(np.asarray(key, dtype=np.float32)).reshape(B, S, C)
    value = np.ascontiguousarray(np.asarray(value, dtype=np.float32)).reshape(B, S, C)

    masks = _make_masks()
    pad = np.zeros((NPOS * 128, 2), np.float32)
    pad[:, 0] = 1.0
    in_maps = []
    for core in range(8):
        b, h = core // 2, core % 2
        k_sel = key[b].reshape(NBLK, 128, C)[h::2].reshape(NPOS * 128, C)
        v_sel = value[b].reshape(NBLK, 128, C)[h::2].reshape(NPOS * 128, C)
        in_maps.append(
            {
                "qt_in": np.ascontiguousarray(query[b].T.astype(BF)),
                "kt_in": np.ascontiguousarray(k_sel.T.astype(BF)),
                "v_in": np.ascontiguousarray(
                    np.concatenate([v_sel, pad], axis=1).astype(BF)
                ),
                "m_in": masks[h],
            }
        )

    nc = _get_nc()
    res = run_bass_kernel_spmd(
        nc,
        in_maps,
        list(range(8)),
        trace=trace,
        trace_cores=trace_cores,
    )

    out = np.empty((B, S, C), np.float32)
    for b in range(B):
        o0 = np.asarray(res.results[2 * b]["o_out"]).astype(np.float64)
        o1 = np.asarray(res.results[2 * b + 1]["o_out"]).astype(np.float64)
        num = o0[:, :C] + o1[:, :C]
        den = o0[:, C] + o1[:, C]
        den = np.where(den == 0.0, 1.0, den)
        out[b] = (num / den[:, None]).astype(np.float32)
    return out.reshape(B, 64, 64, C), res


def kernel(query, key, value):
    out, _ = _run(query, key, value, trace=False)
    return out


# revision 27
# speedup vs baseline: 1.0089x; 1.0089x over previous
"""Trainium2 Bass kernel for PixelSNAIL-style strict-causal attention.

Problem: query/key/value [B=4, H=64, W=64, C=256] fp32.
  S = 4096 tokens per batch; scores = (Q K^T)/16 with strict causal mask
  (position i attends to j < i); out = softmax(scores) @ V (row 0 -> 0).

Strategy (8 NeuronCores):
  - 2 cores per batch: context-parallel split of the key/value blocks by
    parity (core h owns k-blocks h, h+2, ..., h+30). Every core runs the
    IDENTICAL program (SPMD) over all 32 query blocks of its batch.
  - No max-subtraction in softmax (scores ~ N(0,1), exp is safe in fp32),
    so per-core partial numerators/denominators combine exactly on host.
  - Host pre-transposes Q and K (c-major) and converts Q/K/V to bf16:
    halves HBM traffic (input stream 23.6us -> 12.9us), bf16 weights get
    fast-weight-load so LDWEIGHTS hides under shorter streams, and
    measured per-matmul issue at equal chip clock is slightly better than
    fp32r. Matmuls accumulate fp32 in PSUM; quantization error ~2e-3 rel,
    far under the 2e-2 gate. V gets a ones-column so the softmax
    denominator accumulates in PSUM alongside the numerator.
  - Strict-causal/diagonal masking is data-driven: an fp32 additive mask
    applied only on each q-slot's diagonal position-pair. The mask rides
    the scalar HWDGE queue FIRST so it lands ~8us in (a late mask stalls
    slot 7's diagonal pair and re-throttles the PE clock gate).
  - Output is bf16 (halves the output drain): per slot the four PSUM O
    tiles drain via VectorE (qs 0,1) + ScalarE (qs 2,3) into one bf16
    SBUF tile shipped by a single 3D-AP DMA on sync.

Layout per core (b = core//2, h = core%2):
  qt_in [256, 4096] bf16 = Q[b]^T
  kt_in [256, 2048] bf16 = K[b][blocks h::2]^T
  v_in  [2048, 258] bf16 = V[b][blocks h::2] ++ ones column
  m_in  [128, 768] fp32  = additive mask for the diagonal position-pair
  o_out [4096, 258] bf16 = partial (numerator ++ denominator) for this core

Program: slot order 7,6,5,4,3,2,0,1 -- descending so the compute-heavy
  slot overlaps the input streaming, with the two tiny slots (0: 1 pair,
  1: 2 pairs) sandwiched after slot 2: their serial St->mask->exp latency
  is absorbed by the neighbouring slots' PE work instead of stalling a
  bare pipeline tail. Big slots (p>=5) iterate pairs in kt-arrival order
  (diagonal last); small slots run the diagonal pair first (its longer
  chain hides the previous slot's O-bank drain). A bf16 warmup burst plus
  filler matmuls in slot 7 keep the PE clock gate open through the
  DMA-paced opening.
"""

import numpy as np

B = 4
S = 4096          # 64*64 tokens per batch
C = 256
NBLK = 32         # 128-row k blocks per batch
NPOS = 16         # k blocks per core (parity split)
NSLOT = 8         # q slots of 512 rows
SCALE = 1.0 / 16.0
NEG = -1.0e30

_CACHE = {}


def _build_nc():
    import concourse.bacc as bacc
    import concourse.mybir as mybir
    import concourse.tile as tile

    F32 = mybir.dt.float32
    BF16 = mybir.dt.bfloat16

    nc = bacc.Bacc("TRN2", target_bir_lowering=False, debug=False, num_devices=8)
    qt_in = nc.dram_tensor("qt_in", [C, S], BF16, kind="ExternalInput").ap()
    kt_in = nc.dram_tensor("kt_in", [C, NPOS * 128], BF16, kind="ExternalInput").ap()
    v_in = nc.dram_tensor("v_in", [NPOS * 128, 258], BF16, kind="ExternalInput").ap()
    m_in = nc.dram_tensor("m_in", [128, 768], F32, kind="ExternalInput").ap()
    o_out = nc.dram_tensor("o_out", [S, 258], BF16, kind="ExternalOutput").ap()

    with tile.TileContext(nc) as tc:
        with (
            tc.tile_pool(name="const", bufs=1) as const,
            tc.tile_pool(name="pt", bufs=3) as ptp,
            tc.tile_pool(name="ms", bufs=2) as msp,
            tc.tile_pool(name="osb", bufs=2) as osbp,
            tc.tile_pool(name="st", bufs=2, space="PSUM") as stp,
            tc.tile_pool(name="op", bufs=4, space="PSUM") as opp,
        ):
            # PE warmup: ~8us of tiny matmuls on memset data, issued during
            # the DMA preamble so the HAM clock gate opens before real work.
            wu = const.tile([128, 64], BF16, tag="wu")
            nc.gpsimd.memset(wu[:], 0.0)
            wu_ps = stp.tile([128, 512], F32, tag="st", name="wu_ps")
            for _ in range(75):
                nc.tensor.matmul(wu_ps[0:64, 0:64], lhsT=wu[:], rhs=wu[:],
                                 start=True, stop=True)

            qt = [const.tile([128, S], BF16, tag=f"qt{c}", name=f"qt{c}") for c in range(2)]
            kt = [
                const.tile([128, NPOS * 128], BF16, tag=f"kt{c}", name=f"kt{c}")
                for c in range(2)
            ]
            vsb = const.tile([128, NPOS * 258], BF16, tag="v")
            mask = const.tile([128, 768], F32, tag="m")

            # DMA placement: qt/kt ride the sync HWDGE ring in consumption
            # order (first-pair chunks first); mask + v pair 0 ride the
            # scalar HWDGE ring (fast, and ACT's first exp isn't until
            # ~12us); the remaining v positions ride gpsimd SWDGE.
            # Per-slot outputs ride sync, queued behind the inputs.
            def qt_dma(c, c0, c1):
                nc.sync.dma_start(qt[c][:, c0:c1], qt_in[c * 128:(c + 1) * 128, c0:c1])

            def kt_dma(c, c0, c1):
                nc.sync.dma_start(kt[c][:, c0:c1], kt_in[c * 128:(c + 1) * 128, c0:c1])

            def v_dma(eng, pos, npos):
                # npos positions in one trigger via a 3D AP
                eng.dma_start(
                    vsb[:, pos * 258:(pos + npos) * 258].rearrange(
                        "p (t v) -> p t v", t=npos
                    ),
                    v_in[pos * 128:(pos + npos) * 128, :].rearrange(
                        "(t p) v -> p t v", p=128
                    ),
                )

            # First-pair loads split across BOTH HWDGE rings so their
            # trigger+receipt latencies run in parallel: kt pair 0 + v pair
            # 0 on scalar, slot-7 queries first on sync. Mask rides gpsimd
            # (first trigger there; not needed until ~25us).
            for c in range(2):
                nc.scalar.dma_start(
                    kt[c][:, 0:256], kt_in[c * 128:(c + 1) * 128, 0:256]
                )                                     # k pair 0
            for c in range(2):
                qt_dma(c, 7 * 512, 8 * 512)           # slot 7 queries
            v_dma(nc.scalar, 0, 2)
            nc.gpsimd.dma_start(mask[:], m_in[:])
            for c in range(2):
                kt_dma(c, 256, 768)                   # k pairs 1-2
            for c in range(2):
                kt_dma(c, 768, 2048)                  # k pairs 3-7
            v_dma(nc.scalar, 2, 2)
            v_dma(nc.scalar, 4, 4)
            for c in range(2):
                qt_dma(c, 6 * 512, 7 * 512)           # slot 6 queries
            v_dma(nc.gpsimd, 8, 8)
            for c in range(2):
                qt_dma(c, 4 * 512, 6 * 512)           # slots 5-4
            for c in range(2):
                qt_dma(c, 0, 4 * 512)                 # slots 3-0

            def emit_drain(o_ps, p):
                # O drain: split across VectorE and ScalarE into one bf16
                # SBUF tile, shipped by 3D-AP DMA(s) on sync. Emission is
                # DEFERRED into the next slot (after its diagonal pair's
                # mask-add + exp) so the mask TENSOR_TENSOR sits at the head
                # of the strict-FIFO DVE queue instead of behind these casts.
                ob = osbp.tile([128, 4 * 258], BF16, tag="ob", name=f"ob{p}")
                for qs in range(4):
                    eng = nc.vector.tensor_copy if qs < 2 else nc.scalar.copy
                    eng(ob[:, qs * 258:(qs + 1) * 258], o_ps[qs][:])
                if p == 1:
                    # Final slot processed: ship each half as soon as its
                    # copy engine finishes, so the last (small) DMA's HBM
                    # write receipt starts earlier.
                    for half in range(2):
                        nc.sync.dma_start(
                            o_out[p * 512 + half * 256:p * 512 + (half + 1) * 256, :]
                            .rearrange("(t p) v -> p t v", p=128),
                            ob[:, half * 516:(half + 1) * 516].rearrange(
                                "p (t v) -> p t v", t=2
                            ),
                        )
                else:
                    nc.sync.dma_start(
                        o_out[p * 512:(p + 1) * 512, :].rearrange(
                            "(t p) v -> p t v", p=128
                        ),
                        ob[:].rearrange("p (t v) -> p t v", t=4),
                    )

            pending_drain = None
            for p in (7, 6, 5, 4, 3, 2, 0, 1):
                o_ps = None
                # Slot 7 (DMA-paced) iterates pairs in kt-arrival order with
                # the masked pair last; all later slots run the masked pair
                # first so its longer St->mask->exp chain overlaps the
                # previous slot's O-bank drain.
                if p == NSLOT - 1:
                    t_order = list(range(p + 1))
                else:
                    t_order = [p] + list(range(p))
                for ti, t in enumerate(t_order):
                    last = t == p
                    first_it = ti == 0
                    last_it = ti == len(t_order) - 1
                    # On the masked pair, position jp=1 is fully blocked for
                    # q-sub-blocks 0,1 on both cores: compute only the live
                    # 256-column half.
                    width = 768 if last else 1024
                    st = stp.tile([128, 1024], F32, tag="st", name=f"st{p}_{t}")
                    if p == NSLOT - 1 and t > 0:
                        # Keep the PE clock gate open during the DMA-paced
                        # streaming phase: tiny filler matmuls into this pair's
                        # St region, overwritten by the real start=True matmul.
                        # Pair 0 gets NONE: its data lands right as the warmup
                        # burst ends, and fillers ahead of it in the in-order
                        # tensor queue only delay the first real matmul.
                        for _ in range(24 if t < 3 else 8):
                            nc.tensor.matmul(
                                st[0:64, 0:64], lhsT=wu[:], rhs=wu[:],
                                start=True, stop=True,
                            )
                    for jp in range(2):
                        pos = 2 * t + jp
                        qoff = p * 512 + (256 if (last and jp == 1) else 0)
                        n = 256 if (last and jp == 1) else 512
                        for c in range(2):
                            nc.tensor.matmul(
                                st[:, jp * 512:jp * 512 + n],
                                lhsT=kt[c][:, pos * 128:(pos + 1) * 128],
                                rhs=qt[c][:, qoff:qoff + n],
                                start=(c == 0),
                                stop=(c == 1),
                            )
                    pt = ptp.tile([128, 1024], BF16, tag="pt", name=f"pt{p}_{t}")
                    if last:
                        # Masked-score sum goes to SBUF scratch, not back
                        # into the St PSUM bank: a same-bank read+write on
                        # DVE runs at half rate (~966ns for 768 cols vs
                        # ~490ns to SBUF), and this add sits on the slot
                        # boundary critical path.
                        ms = msp.tile([128, 768], F32, tag="ms", name=f"ms{p}")
                        nc.vector.tensor_tensor(
                            ms[:, :width], st[:, :width], mask[:, :width],
                            mybir.AluOpType.add,
                        )
                        nc.scalar.activation(
                            pt[:, :width], ms[:, :width],
                            mybir.ActivationFunctionType.Exp, scale=SCALE,
                        )
                    else:
                        nc.scalar.activation(
                            pt[:, :width], st[:, :width],
                            mybir.ActivationFunctionType.Exp, scale=SCALE,
                        )
                    if first_it:
                        # Emit the previous slot's deferred drain now (its
                        # copies queue AFTER this pair's mask-add and exp on
                        # the DVE/ACT rings), then allocate this slot's O
                        # tiles so the pool's write-after-read ordering sees
                        # those copies first.
                        if pending_drain is not None:
                            emit_drain(*pending_drain)
                            pending_drain = None
                        o_ps = [
                            opp.tile([128, 258], F32, tag="o", name=f"o_ps{p}_{qs}")
                            for qs in range(4)
                        ]
                    # qs-outer so each o_ps[qs] takes its final write as
                    # early as possible -- its drain copy can then start
                    # before the pair's remaining PV matmuls finish.
                    for qs in range(4):
                        for jp in range(2):
                            if last and jp == 1 and qs < 2:
                                continue
                            loff = jp * 512 + qs * 128
                            if last and jp == 1:
                                loff = 512 + (qs - 2) * 128
                            nc.tensor.matmul(
                                o_ps[qs][:],
                                lhsT=pt[:, loff:loff + 128],
                                rhs=vsb[:, (2 * t + jp) * 258:(2 * t + jp + 1) * 258],
                                start=(first_it and jp == 0),
                                stop=(last_it and jp == (0 if (last and qs < 2) else 1)),
                            )
                pending_drain = (o_ps, p)
            emit_drain(*pending_drain)
    nc.compile()
    return nc


def _get_nc():
    if "nc" not in _CACHE:
        _CACHE["nc"] = _build_nc()
    return _CACHE["nc"]


def _make_masks():
    """Additive masks [128, 768] for the diagonal position-pair of each slot.

    Free-dim layout: (jp in {0,1}) x (qs in {0..3}) x 128. On the diag pair
    t=p, position jp holds k-block 4p + 2*jp + h vs q-sub-block 4p + qs:
      block <  qblock -> fully allowed (0)
      block == qblock -> strict lower-triangular (allowed iff q_local > k_local)
      block >  qblock -> fully blocked (NEG)
    """
    k_loc = np.arange(128)[:, None]
    q_loc = np.arange(128)[None, :]
    strict = np.where(q_loc > k_loc, 0.0, NEG).astype(np.float32)
    zeros = np.zeros((128, 128), np.float32)
    blocked = np.full((128, 128), NEG, np.float32)
    masks = []
    for h in range(2):
        chunks = []
        for jp, qs_list in ((0, (0, 1, 2, 3)), (1, (2, 3))):
            rel = 2 * jp + h  # k-block offset relative to 4p
            for qs in qs_list:
                if rel < qs:
                    chunks.append(zeros)
                elif rel == qs:
                    chunks.append(strict)
                else:
                    chunks.append(blocked)
        masks.append(np.concatenate(chunks, axis=1))
    return masks


def _run(query, key, value, trace=False, trace_cores=None):
    import ml_dtypes
    from concourse.bass_utils import run_bass_kernel_spmd

    BF = ml_dtypes.bfloat16
    query = np.ascontiguousarray(np.asarray(query, dtype=np.float32)).reshape(B, S, C)
    key = np.ascontiguousarray(np.asarray(key, dtype=np.float32)).reshape(B, S, C)
    value = np.ascontiguousarray(np.asarray(value, dtype=np.float32)).reshape(B, S, C)

    masks = _make_masks()
    pad = np.zeros((NPOS * 128, 2), np.float32)
    pad[:, 0] = 1.0
    in_maps = []
    for core in range(8):
        b, h = core // 2, core % 2
        k_sel = key[b].reshape(NBLK, 128, C)[h::2].reshape(NPOS * 128, C)
        v_sel = value[b].reshape(NBLK, 128, C)[h::2].reshape(NPOS * 128, C)
        in_maps.append(
            {
                "qt_in": np.ascontiguousarray(query[b].T.astype(BF)),
                "kt_in": np.ascontiguousarray(k_sel.T.astype(BF)),
                "v_in": np.ascontiguousarray(
                    np.concatenate([v_sel, pad], axis=1).astype(BF)
                ),
                "m_in": masks[h],
            }
        )

    nc = _get_nc()
    res = run_bass_kernel_spmd(
        nc,
        in_maps,
        list(range(8)),
        trace=trace,
        trace_cores=trace_cores,
    )

    out = np.empty((B, S, C), np.float32)
    for b in range(B):
        o0 = np.asarray(res.results[2 * b]["o_out"]).astype(np.float64)
        o1 = np.asarray(res.results[2 * b + 1]["o_out"]).astype(np.float64)
        num = o0[:, :C] + o1[:, :C]
        den = o0[:, C] + o1[:, C]
        den = np.where(den == 0.0, 1.0, den)
        out[b] = (num / den[:, None]).astype(np.float32)
    return out.reshape(B, 64, 64, C), res


def kernel(query, key, value):
    out, _ = _run(query, key, value, trace=False)
    return out


# revision 29
# speedup vs baseline: 1.0362x; 1.0271x over previous
"""Trainium2 Bass kernel for PixelSNAIL-style strict-causal attention.

Problem: query/key/value [B=4, H=64, W=64, C=256] fp32.
  S = 4096 tokens per batch; scores = (Q K^T)/16 with strict causal mask
  (position i attends to j < i); out = softmax(scores) @ V (row 0 -> 0).

Strategy (8 NeuronCores):
  - 2 cores per batch: context-parallel split of the key/value blocks by
    parity (core h owns k-blocks h, h+2, ..., h+30). Every core runs the
    IDENTICAL program (SPMD) over all 32 query blocks of its batch.
  - No max-subtraction in softmax (scores ~ N(0,1), exp is safe in fp32),
    so per-core partial numerators/denominators combine exactly on host.
  - Host pre-transposes Q and K (c-major) and converts Q/K/V to bf16:
    halves HBM traffic (input stream 23.6us -> 12.9us), bf16 weights get
    fast-weight-load so LDWEIGHTS hides under shorter streams, and
    measured per-matmul issue at equal chip clock is slightly better than
    fp32r. Matmuls accumulate fp32 in PSUM; quantization error ~2e-3 rel,
    far under the 2e-2 gate. V gets a ones-column so the softmax
    denominator accumulates in PSUM alongside the numerator.
  - Strict-causal/diagonal masking is data-driven: an fp32 additive mask
    applied only on each q-slot's diagonal position-pair. The mask rides
    the scalar HWDGE queue FIRST so it lands ~8us in (a late mask stalls
    slot 7's diagonal pair and re-throttles the PE clock gate).
  - Output is bf16 (halves the output drain): per slot the four PSUM O
    tiles drain via VectorE (qs 0,1) + ScalarE (qs 2,3) into one bf16
    SBUF tile shipped by a single 3D-AP DMA on sync.

Layout per core (b = core//2, h = core%2):
  qt_in [256, 4096] bf16 = Q[b]^T
  kt_in [256, 2048] bf16 = K[b][blocks h::2]^T
  v_in  [2048, 258] bf16 = V[b][blocks h::2] ++ ones column
  m_in  [128, 768] fp32  = additive mask for the diagonal position-pair
  o_out [4096, 258] bf16 = partial (numerator ++ denominator) for this core

Program: slot order 7,6,5,4,3,2,0,1 -- descending so the compute-heavy
  slot overlaps the input streaming, with the two tiny slots (0: 1 pair,
  1: 2 pairs) sandwiched after slot 2: their serial St->mask->exp latency
  is absorbed by the neighbouring slots' PE work instead of stalling a
  bare pipeline tail. Big slots (p>=5) iterate pairs in kt-arrival order
  (diagonal last); small slots run the diagonal pair first (its longer
  chain hides the previous slot's O-bank drain). A bf16 warmup burst plus
  filler matmuls in slot 7 keep the PE clock gate open through the
  DMA-paced opening.
"""

import numpy as np

B = 4
S = 4096          # 64*64 tokens per batch
C = 256
NBLK = 32         # 128-row k blocks per batch
NPOS = 16         # k blocks per core (parity split)
NSLOT = 8         # q slots of 512 rows
SCALE = 1.0 / 16.0
NEG = -1.0e30

_CACHE = {}


def _build_nc():
    import concourse.bacc as bacc
    import concourse.mybir as mybir
    import concourse.tile as tile

    F32 = mybir.dt.float32
    BF16 = mybir.dt.bfloat16

    nc = bacc.Bacc("TRN2", target_bir_lowering=False, debug=False, num_devices=8)
    qt_in = nc.dram_tensor("qt_in", [C, S], BF16, kind="ExternalInput").ap()
    kt_in = nc.dram_tensor("kt_in", [C, NPOS * 128], BF16, kind="ExternalInput").ap()
    v_in = nc.dram_tensor("v_in", [NPOS * 128, 258], BF16, kind="ExternalInput").ap()
    m_in = nc.dram_tensor("m_in", [128, 768], F32, kind="ExternalInput").ap()
    o_out = nc.dram_tensor("o_out", [S, 258], BF16, kind="ExternalOutput").ap()

    with tile.TileContext(nc) as tc:
        with (
            tc.tile_pool(name="const", bufs=1) as const,
            tc.tile_pool(name="pt", bufs=3) as ptp,
            tc.tile_pool(name="ms", bufs=2) as msp,
            tc.tile_pool(name="osb", bufs=2) as osbp,
            tc.tile_pool(name="st", bufs=2, space="PSUM") as stp,
            tc.tile_pool(name="op", bufs=4, space="PSUM") as opp,
        ):
            # PE warmup: ~8us of tiny matmuls on memset data, issued during
            # the DMA preamble so the HAM clock gate opens before real work.
            wu = const.tile([128, 64], BF16, tag="wu")
            nc.gpsimd.memset(wu[:], 0.0)
            wu_ps = stp.tile([128, 512], F32, tag="st", name="wu_ps")
            for _ in range(75):
                nc.tensor.matmul(wu_ps[0:64, 0:64], lhsT=wu[:], rhs=wu[:],
                                 start=True, stop=True)

            qt = [const.tile([128, S], BF16, tag=f"qt{c}", name=f"qt{c}") for c in range(2)]
            kt = [
                const.tile([128, NPOS * 128], BF16, tag=f"kt{c}", name=f"kt{c}")
                for c in range(2)
            ]
            vsb = const.tile([128, NPOS * 258], BF16, tag="v")
            mask = const.tile([128, 768], F32, tag="m")

            # DMA placement: qt/kt ride the sync HWDGE ring in consumption
            # order (first-pair chunks first); mask + v pair 0 ride the
            # scalar HWDGE ring (fast, and ACT's first exp isn't until
            # ~12us); the remaining v positions ride gpsimd SWDGE.
            # Per-slot outputs ride sync, queued behind the inputs.
            def qt_dma(c, c0, c1):
                nc.sync.dma_start(qt[c][:, c0:c1], qt_in[c * 128:(c + 1) * 128, c0:c1])

            def kt_dma(c, c0, c1):
                nc.sync.dma_start(kt[c][:, c0:c1], kt_in[c * 128:(c + 1) * 128, c0:c1])

            def v_dma(eng, pos, npos):
                # npos positions in one trigger via a 3D AP
                eng.dma_start(
                    vsb[:, pos * 258:(pos + npos) * 258].rearrange(
                        "p (t v) -> p t v", t=npos
                    ),
                    v_in[pos * 128:(pos + npos) * 128, :].rearrange(
                        "(t p) v -> p t v", p=128
                    ),
                )

            # First-pair loads split across BOTH HWDGE rings so their
            # trigger+receipt latencies run in parallel: kt pair 0 + v pair
            # 0 on scalar, slot-7 queries first on sync. Mask rides gpsimd
            # (first trigger there; not needed until ~25us).
            for c in range(2):
                nc.scalar.dma_start(
                    kt[c][:, 0:256], kt_in[c * 128:(c + 1) * 128, 0:256]
                )                                     # k pair 0
            # c-interleaved so each c-chunk's consumers unblock as it lands:
            # pair 0/1 score matmuls below run c-outer to match.
            qt_dma(0, 7 * 512, 8 * 512)               # slot 7 queries c0
            kt_dma(0, 256, 768)                       # k pairs 1-2 c0
            qt_dma(1, 7 * 512, 8 * 512)               # slot 7 queries c1
            kt_dma(1, 256, 768)                       # k pairs 1-2 c1
            v_dma(nc.scalar, 0, 2)
            nc.gpsimd.dma_start(mask[:], m_in[:])
            for c in range(2):
                kt_dma(c, 768, 2048)                  # k pairs 3-7
            v_dma(nc.scalar, 2, 2)
            v_dma(nc.scalar, 4, 4)
            for c in range(2):
                qt_dma(c, 6 * 512, 7 * 512)           # slot 6 queries
            v_dma(nc.gpsimd, 8, 8)
            for c in range(2):
                qt_dma(c, 4 * 512, 6 * 512)           # slots 5-4
            for c in range(2):
                qt_dma(c, 0, 4 * 512)                 # slots 3-0

            def emit_drain(o_ps, p):
                # O drain: split across VectorE and ScalarE into one bf16
                # SBUF tile, shipped by 3D-AP DMA(s) on sync. Emission is
                # DEFERRED into the next slot (after its diagonal pair's
                # mask-add + exp) so the mask TENSOR_TENSOR sits at the head
                # of the strict-FIFO DVE queue instead of behind these casts.
                ob = osbp.tile([128, 4 * 258], BF16, tag="ob", name=f"ob{p}")
                for qs in range(4):
                    eng = nc.vector.tensor_copy if qs < 2 else nc.scalar.copy
                    eng(ob[:, qs * 258:(qs + 1) * 258], o_ps[qs][:])
                if p == 1:
                    # Final slot processed: ship each half as soon as its
                    # copy engine finishes, so the last (small) DMA's HBM
                    # write receipt starts earlier.
                    for half in range(2):
                        nc.sync.dma_start(
                            o_out[p * 512 + half * 256:p * 512 + (half + 1) * 256, :]
                            .rearrange("(t p) v -> p t v", p=128),
                            ob[:, half * 516:(half + 1) * 516].rearrange(
                                "p (t v) -> p t v", t=2
                            ),
                        )
                else:
                    nc.sync.dma_start(
                        o_out[p * 512:(p + 1) * 512, :].rearrange(
                            "(t p) v -> p t v", p=128
                        ),
                        ob[:].rearrange("p (t v) -> p t v", t=4),
                    )

            pending_drain = None
            for p in (7, 6, 5, 4, 3, 2, 0, 1):
                o_ps = None
                # Slot 7 (DMA-paced) iterates pairs in kt-arrival order with
                # the masked pair last; all later slots run the masked pair
                # first so its longer St->mask->exp chain overlaps the
                # previous slot's O-bank drain.
                if p == NSLOT - 1:
                    t_order = list(range(p + 1))
                else:
                    t_order = [p] + list(range(p))
                for ti, t in enumerate(t_order):
                    last = t == p
                    first_it = ti == 0
                    last_it = ti == len(t_order) - 1
                    # On the masked pair, position jp=1 is fully blocked for
                    # q-sub-blocks 0,1 on both cores: compute only the live
                    # 256-column half.
                    width = 768 if last else 1024
                    st = stp.tile([128, 1024], F32, tag="st", name=f"st{p}_{t}")
                    if p == NSLOT - 1 and t > 0:
                        # Keep the PE clock gate open during the DMA-paced
                        # streaming phase: tiny filler matmuls into this pair's
                        # St region, overwritten by the real start=True matmul.
                        # Pair 0 gets NONE: its data lands right as the warmup
                        # burst ends, and fillers ahead of it in the in-order
                        # tensor queue only delay the first real matmul.
                        for _ in range(24 if t < 3 else 8):
                            nc.tensor.matmul(
                                st[0:64, 0:64], lhsT=wu[:], rhs=wu[:],
                                start=True, stop=True,
                            )
                    # DMA-paced opening pairs run c-outer (both jp halves of
                    # c0 before any c1) so the PE streams c0 work while the
                    # c1 chunks are still in flight. Accumulation semantics
                    # are unchanged: each St region still gets c0 with
                    # start=True, then c1 with stop=True.
                    if p == NSLOT - 1 and t <= 1:
                        cjp_order = [(c, jp) for c in range(2) for jp in range(2)]
                    else:
                        cjp_order = [(c, jp) for jp in range(2) for c in range(2)]
                    for c, jp in cjp_order:
                        pos = 2 * t + jp
                        qoff = p * 512 + (256 if (last and jp == 1) else 0)
                        n = 256 if (last and jp == 1) else 512
                        nc.tensor.matmul(
                            st[:, jp * 512:jp * 512 + n],
                            lhsT=kt[c][:, pos * 128:(pos + 1) * 128],
                            rhs=qt[c][:, qoff:qoff + n],
                            start=(c == 0),
                            stop=(c == 1),
                        )
                    pt = ptp.tile([128, 1024], BF16, tag="pt", name=f"pt{p}_{t}")
                    if last:
                        # Masked-score sum goes to SBUF scratch, not back
                        # into the St PSUM bank: a same-bank read+write on
                        # DVE runs at half rate (~966ns for 768 cols vs
                        # ~490ns to SBUF), and this add sits on the slot
                        # boundary critical path.
                        ms = msp.tile([128, 768], F32, tag="ms", name=f"ms{p}")
                        nc.vector.tensor_tensor(
                            ms[:, :width], st[:, :width], mask[:, :width],
                            mybir.AluOpType.add,
                        )
                        nc.scalar.activation(
                            pt[:, :width], ms[:, :width],
                            mybir.ActivationFunctionType.Exp, scale=SCALE,
                        )
                    else:
                        nc.scalar.activation(
                            pt[:, :width], st[:, :width],
                            mybir.ActivationFunctionType.Exp, scale=SCALE,
                        )
                    if first_it:
                        # Emit the previous slot's deferred drain now (its
                        # copies queue AFTER this pair's mask-add and exp on
                        # the DVE/ACT rings), then allocate this slot's O
                        # tiles so the pool's write-after-read ordering sees
                        # those copies first.
                        if pending_drain is not None:
                            emit_drain(*pending_drain)
                            pending_drain = None
                        o_ps = [
                            opp.tile([128, 258], F32, tag="o", name=f"o_ps{p}_{qs}")
                            for qs in range(4)
                        ]
                    # qs-outer so each o_ps[qs] takes its final write as
                    # early as possible -- its drain copy can then start
                    # before the pair's remaining PV matmuls finish.
                    for qs in range(4):
                        for jp in range(2):
                            if last and jp == 1 and qs < 2:
                                continue
                            loff = jp * 512 + qs * 128
                            if last and jp == 1:
                                loff = 512 + (qs - 2) * 128
                            nc.tensor.matmul(
                                o_ps[qs][:],
                                lhsT=pt[:, loff:loff + 128],
                                rhs=vsb[:, (2 * t + jp) * 258:(2 * t + jp + 1) * 258],
                                start=(first_it and jp == 0),
                                stop=(last_it and jp == (0 if (last and qs < 2) else 1)),
                            )
                pending_drain = (o_ps, p)
            emit_drain(*pending_drain)
    nc.compile()
    return nc


def _get_nc():
    if "nc" not in _CACHE:
        _CACHE["nc"] = _build_nc()
    return _CACHE["nc"]


def _make_masks():
    """Additive masks [128, 768] for the diagonal position-pair of each slot.

    Free-dim layout: (jp in {0,1}) x (qs in {0..3}) x 128. On the diag pair
    t=p, position jp holds k-block 4p + 2*jp + h vs q-sub-block 4p + qs:
      block <  qblock -> fully allowed (0)
      block == qblock -> strict lower-triangular (allowed iff q_local > k_local)
      block >  qblock -> fully blocked (NEG)
    """
    k_loc = np.arange(128)[:, None]
    q_loc = np.arange(128)[None, :]
    strict = np.where(q_loc > k_loc, 0.0, NEG).astype(np.float32)
    zeros = np.zeros((128, 128), np.float32)
    blocked = np.full((128, 128), NEG, np.float32)
    masks = []
    for h in range(2):
        chunks = []
        for jp, qs_list in ((0, (0, 1, 2, 3)), (1, (2, 3))):
            rel = 2 * jp + h  # k-block offset relative to 4p
            for qs in qs_list:
                if rel < qs:
                    chunks.append(zeros)
                elif rel == qs:
                    chunks.append(strict)
                else:
                    chunks.append(blocked)
        masks.append(np.concatenate(chunks, axis=1))
    return masks


def _run(query, key, value, trace=False, trace_cores=None):
    import ml_dtypes
    from concourse.bass_utils import run_bass_kernel_spmd

    BF = ml_dtypes.bfloat16
    query = np.ascontiguousarray(np.asarray(query, dtype=np.float32)).reshape(B, S, C)
    key = np.ascontiguousarray(np.asarray(key, dtype=np.float32)).reshape(B, S, C)
    value = np.ascontiguousarray(np.asarray(value, dtype=np.float32)).reshape(B, S, C)

    masks = _make_masks()
    pad = np.zeros((NPOS * 128, 2), np.float32)
    pad[:, 0] = 1.0
    in_maps = []
    for core in range(8):
        b, h = core // 2, core % 2
        k_sel = key[b].reshape(NBLK, 128, C)[h::2].reshape(NPOS * 128, C)
        v_sel = value[b].reshape(NBLK, 128, C)[h::2].reshape(NPOS * 128, C)
        in_maps.append(
            {
                "qt_in": np.ascontiguousarray(query[b].T.astype(BF)),
                "kt_in": np.ascontiguousarray(k_sel.T.astype(BF)),
                "v_in": np.ascontiguousarray(
                    np.concatenate([v_sel, pad], axis=1).astype(BF)
                ),
                "m_in": masks[h],
            }
        )

    nc = _get_nc()
    res = run_bass_kernel_spmd(
        nc,
        in_maps,
        list(range(8)),
        trace=trace,
        trace_cores=trace_cores,
    )

    out = np.empty((B, S, C), np.float32)
    for b in range(B):
        o0 = np.asarray(res.results[2 * b]["o_out"]).astype(np.float64)
        o1 = np.asarray(res.results[2 * b + 1]["o_out"]).astype(np.float64)
        num = o0[:, :C] + o1[:, :C]
        den = o0[:, C] + o1[:, C]
        den = np.where(den == 0.0, 1.0, den)
        out[b] = (num / den[:, None]).astype(np.float32)
    return out.reshape(B, 64, 64, C), res


def kernel(query, key, value):
    out, _ = _run(query, key, value, trace=False)
    return out
